# revision 7
# baseline (speedup 1.0000x reference)
"""Trainium2 Bass kernel for CrossInnerProductWithBuyer.

Computes, per batch b (B=16384, E=128):
  out[b] = concat( windows[b] @ c[b],      # [10]
                   -(neg[b] @ c[b]),       # [64]
                   buy[b] @ c[b] )         # [1]
with c = center_vec.  Output [B, 75, 1] fp32.

Memory-bound problem: per core ~80 MB fp32 of input, ~0.6 MB output.
The host casts inputs to fp16 (verified max-rel-err ~4e-4, gate 2e-2),
halving HBM traffic -> DMA floor ~110 us per core.

Layout (per core, bs=2048 batches, 16 tiles of PT=128 batches):
  at [E=128, nt*F]  fp16, per tile F = 76*128 cols ordered (r, b):
                    r in win(10)|neg(64)|buy(1)|pad(1), b innermost.
  ct [E=128, bs]    fp16 transposed center vectors.

Per tile the kernel does:
  - DVE: ONE in-place tensor_mul against c broadcast over the middle
    r axis (b innermost, step 1, fp16 -> 2x_1P DVE mode).  The r
    order is [win|buy|pad|neg] so every 512-col chunk is sign-pure;
    the neg sign is folded into a minus-ones matmul stationary.
  - PE:  (+-)ones[128,1]-stationary fp16 matmuls reduce e over the
    partition axis, 512 cols per matmul.  Strip s goes to PSUM
    partition 32*((s%12)//4), bank s%4 -- 12 strips fill a
    [128,2048] 4-bank PSUM mega-tile on partitions {0,32,64}.
  - ACT: one [128,2048] PSUM->SBUF copy per mega-tile (~2us for
    12 strips instead of per-strip single-partition copies).
  - Sync engine: input DMAs and 3 output DMAs of 8 KB per mega-tile
    (issuing out-DMAs from ACT would cost ~0.65us each there).

Host post-pass re-transposes [t,k,r4,b] -> [b,r], moves buy back
behind neg, and drops the pad col.
"""

import sys

if "/opt/trn_rl_repo" not in sys.path:
    sys.path.insert(0, "/opt/trn_rl_repo")

from contextlib import ExitStack

import numpy as np

import concourse.bass as bass
import concourse.mybir as mybir
import concourse.tile as tile
from concourse import bacc, bass_utils

B, W, N, E = 16384, 10, 64, 128
NCORES = 8
BS = B // NCORES            # 2048 batches per core
PT = 128                    # batches per tile
NT = BS // PT               # 16 tiles
R = W + N + 1               # 75 real output rows per batch
RP = 76                     # padded (one zero row) -> 19 chunks of 512
F = RP * PT                 # 9728 cols per tile
CHUNK = 512                 # matmul N (one PSUM bank of fp32)
NK = F // CHUNK             # 19 chunks per tile
NSTRIP = NT * NK            # 304 strips per core
NMEGA = NSTRIP // 16        # 19 PSUM mega-tiles (16 strips each)

FP32 = mybir.dt.float32
FP16 = mybir.dt.float16


def _build(bs: int = BS) -> bass.Bass:
    nt = bs // PT
    nstrip = nt * NK
    nmega = nstrip // 16
    nc = bacc.Bacc("TRN2", target_bir_lowering=False, debug=False,
                   num_devices=NCORES)
    at = nc.dram_tensor("at", [E, nt * F], FP16, kind="ExternalInput").ap()
    ct = nc.dram_tensor("ct", [E, bs], FP16, kind="ExternalInput").ap()
    out = nc.dram_tensor("out", [nstrip // 4, 4 * CHUNK], FP32,
                         kind="ExternalOutput").ap()

    with tile.TileContext(nc) as tc, ExitStack() as ctx:
        apool = ctx.enter_context(tc.tile_pool(name="a", bufs=5))
        cpool = ctx.enter_context(tc.tile_pool(name="c", bufs=1))
        spool = ctx.enter_context(tc.tile_pool(name="strip", bufs=6))
        pspool = ctx.enter_context(tc.tile_pool(name="ps", bufs=2,
                                                space="PSUM"))
        onepool = ctx.enter_context(tc.tile_pool(name="ones", bufs=1))

        ones = onepool.tile([E, 1], FP16)
        nc.vector.memset(ones[:], 1.0)
        mones = onepool.tile([E, 1], FP16)
        nc.vector.memset(mones[:], -1.0)
        # chunks 0-2 of each tile are win|buy|pad (+1); 3-18 are neg (-1)
        stat = [ones if k < 3 else mones for k in range(NK)]

        c_all = cpool.tile([E, bs], FP16)
        nc.sync.dma_start(c_all[:], ct[:, :])

        a_tiles = [None] * nt
        mega = None
        for s in range(nstrip):
            t, k = divmod(s, NK)
            if k == 0:
                a = apool.tile([E, F], FP16)
                nc.sync.dma_start(a[:], at[:, t * F:(t + 1) * F])
                a_tiles[t] = a
                av = a[:].rearrange("e (r b) -> e r b", b=PT)
                cb = c_all[:, t * PT:(t + 1) * PT]
                nc.vector.tensor_mul(
                    av[:, :, :], av[:, :, :],
                    cb.unsqueeze(1).broadcast_to([E, RP, PT]))

            # 12 strips per PSUM mega-tile: partition 32*j (j in 0..2,
            # AP base partition must be one of {0,32,64}), bank cb_idx.
            m, r12 = divmod(s, 12)
            j, cb_idx = divmod(r12, 4)
            if r12 == 0:
                mega = pspool.tile([E, 4 * CHUNK], FP32)
            nc.tensor.matmul(
                mega[32 * j:32 * j + 1,
                     cb_idx * CHUNK:(cb_idx + 1) * CHUNK],
                stat[k][:],
                a_tiles[t][:, k * CHUNK:(k + 1) * CHUNK],
                start=True, stop=True)
            if r12 == 11 or s == nstrip - 1:
                strip = spool.tile([E, 4 * CHUNK], FP32)
                nc.scalar.copy(strip[:], mega[:])
                for jj in range(j + 1):
                    nc.sync.dma_start(
                        out[3 * m + jj:3 * m + jj + 1, :],
                        strip[32 * jj:32 * jj + 1, :])
    nc.compile()
    return nc


_NC_CACHE: dict = {}


def _get_nc(bs: int = BS) -> bass.Bass:
    if bs not in _NC_CACHE:
        _NC_CACHE[bs] = _build(bs)
    return _NC_CACHE[bs]


def _prep_core(center, windows, negs, buy):
    """Build one core's fp16 (e, t, r, b)-ordered blob + transposed c."""
    bs = center.shape[0]
    nt = bs // PT
    # r order [win(0:10), buy(10), pad(11), neg(12:76)] makes every
    # 512-col chunk sign-pure (chunks 0-2 positive, 3-18 negative).
    a16 = np.zeros((bs, RP, E), np.float16)
    a16[:, :W] = windows
    a16[:, W] = buy[:, 0]
    a16[:, W + 2:] = negs
    # [t, b, r, e] -> [e, t, r, b]
    at = np.ascontiguousarray(
        a16.reshape(nt, PT, RP, E).transpose(3, 0, 2, 1)).reshape(E, nt * F)
    ct = np.ascontiguousarray(center.reshape(bs, E).astype(np.float16).T)
    return at, ct


def _shard_inputs(center_vec, windows_vecs, neg_vecs, buy_vec):
    in_maps = []
    for i in range(NCORES):
        sl = slice(i * BS, (i + 1) * BS)
        at, ct = _prep_core(center_vec[sl], windows_vecs[sl],
                            neg_vecs[sl], buy_vec[sl])
        in_maps.append({"at": at, "ct": ct})
    return in_maps


def _unpack_core(raw):
    """[nstrip/4, 2048] fp32 -> [BS, R] (win, -neg, buy)."""
    o = raw.reshape(NT, NK, 4, PT).transpose(0, 3, 1, 2).reshape(BS, RP)
    res = np.empty((BS, R), np.float32)
    res[:, :W] = o[:, :W]
    res[:, W:W + N] = o[:, W + 2:]
    res[:, W + N] = o[:, W]
    return res


def run(center_vec, windows_vecs, neg_vecs, buy_vec, trace: bool = False):
    """Run on 8 NeuronCores; returns (full_output, BassKernelResults)."""
    nc = _get_nc()
    in_maps = _shard_inputs(np.asarray(center_vec, dtype=np.float32),
                            np.asarray(windows_vecs, dtype=np.float32),
                            np.asarray(neg_vecs, dtype=np.float32),
                            np.asarray(buy_vec, dtype=np.float32))
    res = bass_utils.run_bass_kernel_spmd(
        nc, in_maps, list(range(NCORES)), trace=trace)
    full = np.concatenate(
        [_unpack_core(res.results[i]["out"]) for i in range(NCORES)], axis=0)
    return full.reshape(B, R, 1), res


def kernel(center_vec, windows_vecs, neg_vecs, buy_vec):
    out, _ = run(center_vec, windows_vecs, neg_vecs, buy_vec)
    return out


# revision 8
# speedup vs baseline: 1.5126x; 1.5126x over previous
"""Trainium2 Bass kernel for CrossInnerProductWithBuyer.

Computes, per batch b (B=16384, E=128):
  out[b] = concat( windows[b] @ c[b],      # [10]
                   -(neg[b] @ c[b]),       # [64]
                   buy[b] @ c[b] )         # [1]
with c = center_vec.  Output [B, 75, 1] fp32.

Memory-bound problem: per core ~80 MB fp32 of input, ~0.6 MB output.
The host casts inputs to fp16 (verified max-rel-err ~4e-4, gate 2e-2),
halving HBM traffic -> DMA floor ~110 us per core.

Layout (per core, bs=2048 batches, 16 tiles of PT=128 batches):
  at [E=128, nt*F]  fp16, per tile F = 76*128 cols ordered (r, b):
                    r in win(10)|neg(64)|buy(1)|pad(1), b innermost.
  ct [E=128, bs]    fp16 transposed center vectors.

Per tile the kernel does:
  - DVE: ONE in-place tensor_mul against c broadcast over the middle
    r axis (b innermost, step 1, fp16 -> 2x_1P DVE mode).  The r
    order is [win|buy|pad|neg] so every 512-col chunk is sign-pure;
    the neg sign is folded into a minus-ones matmul stationary.
  - PE:  (+-)ones[128,1]-stationary fp16 matmuls reduce e over the
    partition axis, 512 cols per matmul.  Strip s goes to PSUM
    partition 32*((s%12)//4), bank s%4 -- 12 strips fill a
    [128,2048] 4-bank PSUM mega-tile on partitions {0,32,64}.
  - ACT: one [128,2048] PSUM->SBUF copy per mega-tile (~2us for
    12 strips instead of per-strip single-partition copies).
  - Sync engine: input DMAs and 3 output DMAs of 8 KB per mega-tile
    (issuing out-DMAs from ACT would cost ~0.65us each there).

Host post-pass re-transposes [t,k,r4,b] -> [b,r], moves buy back
behind neg, and drops the pad col.
"""

import sys

if "/opt/trn_rl_repo" not in sys.path:
    sys.path.insert(0, "/opt/trn_rl_repo")

from contextlib import ExitStack

import numpy as np

import concourse.bass as bass
import concourse.mybir as mybir
import concourse.tile as tile
from concourse import bacc, bass_utils

B, W, N, E = 16384, 10, 64, 128
NCORES = 8
BS = B // NCORES            # 2048 batches per core
PT = 128                    # batches per tile
NT = BS // PT               # 16 tiles
R = W + N + 1               # 75 real output rows per batch
RP = 76                     # padded (one zero row) -> 19 chunks of 512
F = RP * PT                 # 9728 cols per tile
CHUNK = 512                 # matmul N (one PSUM bank of fp32)
NK = F // CHUNK             # 19 chunks per tile
NSTRIP = NT * NK            # 304 strips per core
NMEGA = NSTRIP // 16        # 19 PSUM mega-tiles (16 strips each)

FP32 = mybir.dt.float32
FP16 = mybir.dt.float16


def _build(bs: int = BS) -> bass.Bass:
    nt = bs // PT
    nstrip = nt * NK
    nmega = nstrip // 16
    nc = bacc.Bacc("TRN2", target_bir_lowering=False, debug=False,
                   num_devices=NCORES)
    at = nc.dram_tensor("at", [E, nt * F], FP16, kind="ExternalInput").ap()
    ct = nc.dram_tensor("ct", [E, bs], FP16, kind="ExternalInput").ap()
    out = nc.dram_tensor("out", [nstrip // 4, 4 * CHUNK], FP32,
                         kind="ExternalOutput").ap()

    with tile.TileContext(nc) as tc, ExitStack() as ctx:
        apool = ctx.enter_context(tc.tile_pool(name="a", bufs=5))
        cpool = ctx.enter_context(tc.tile_pool(name="c", bufs=1))
        spool = ctx.enter_context(tc.tile_pool(name="strip", bufs=6))
        pspool = ctx.enter_context(tc.tile_pool(name="ps", bufs=2,
                                                space="PSUM"))
        onepool = ctx.enter_context(tc.tile_pool(name="ones", bufs=1))

        ones = onepool.tile([E, 1], FP16)
        nc.vector.memset(ones[:], 1.0)
        mones = onepool.tile([E, 1], FP16)
        nc.vector.memset(mones[:], -1.0)
        # chunks 0-2 of each tile are win|buy|pad (+1); 3-18 are neg (-1)
        stat = [ones if k < 3 else mones for k in range(NK)]

        c_all = cpool.tile([E, bs], FP16)
        nc.sync.dma_start(c_all[:], ct[:, :])

        a_tiles = [None] * nt
        mega = None
        for s in range(nstrip):
            t, k = divmod(s, NK)
            if k == 0:
                a = apool.tile([E, F], FP16)
                nc.sync.dma_start(a[:], at[:, t * F:(t + 1) * F])
                a_tiles[t] = a
                av = a[:].rearrange("e (r b) -> e r b", b=PT)
                cb = c_all[:, t * PT:(t + 1) * PT]
                nc.vector.tensor_mul(
                    av[:, :, :], av[:, :, :],
                    cb.unsqueeze(1).broadcast_to([E, RP, PT]))

            # 12 strips per PSUM mega-tile: partition 32*j (j in 0..2,
            # AP base partition must be one of {0,32,64}), bank cb_idx.
            m, r12 = divmod(s, 12)
            j, cb_idx = divmod(r12, 4)
            if r12 == 0:
                mega = pspool.tile([E, 4 * CHUNK], FP32)
            nc.tensor.matmul(
                mega[32 * j:32 * j + 1,
                     cb_idx * CHUNK:(cb_idx + 1) * CHUNK],
                stat[k][:],
                a_tiles[t][:, k * CHUNK:(k + 1) * CHUNK],
                start=True, stop=True)
            if r12 == 11 or s == nstrip - 1:
                strip = spool.tile([E, 4 * CHUNK], FP32)
                nc.scalar.copy(strip[:], mega[:])
                # one DMA per mega: partitions {0,32,64} -> 3 DRAM rows
                sv = strip[:].rearrange("(a p) f -> a p f", p=32)
                nc.scalar.dma_start(out[3 * m:3 * m + j + 1, :],
                                    sv[0:j + 1, 0, :])
    nc.compile()
    return nc


_NC_CACHE: dict = {}


def _get_nc(bs: int = BS) -> bass.Bass:
    if bs not in _NC_CACHE:
        _NC_CACHE[bs] = _build(bs)
    return _NC_CACHE[bs]


def _prep_core(center, windows, negs, buy):
    """Build one core's fp16 (e, t, r, b)-ordered blob + transposed c."""
    bs = center.shape[0]
    nt = bs // PT
    # r order [win(0:10), buy(10), pad(11), neg(12:76)] makes every
    # 512-col chunk sign-pure (chunks 0-2 positive, 3-18 negative).
    a16 = np.zeros((bs, RP, E), np.float16)
    a16[:, :W] = windows
    a16[:, W] = buy[:, 0]
    a16[:, W + 2:] = negs
    # [t, b, r, e] -> [e, t, r, b]
    at = np.ascontiguousarray(
        a16.reshape(nt, PT, RP, E).transpose(3, 0, 2, 1)).reshape(E, nt * F)
    ct = np.ascontiguousarray(center.reshape(bs, E).astype(np.float16).T)
    return at, ct


def _shard_inputs(center_vec, windows_vecs, neg_vecs, buy_vec):
    in_maps = []
    for i in range(NCORES):
        sl = slice(i * BS, (i + 1) * BS)
        at, ct = _prep_core(center_vec[sl], windows_vecs[sl],
                            neg_vecs[sl], buy_vec[sl])
        in_maps.append({"at": at, "ct": ct})
    return in_maps


def _unpack_core(raw):
    """[nstrip/4, 2048] fp32 -> [BS, R] (win, -neg, buy)."""
    o = raw.reshape(NT, NK, 4, PT).transpose(0, 3, 1, 2).reshape(BS, RP)
    res = np.empty((BS, R), np.float32)
    res[:, :W] = o[:, :W]
    res[:, W:W + N] = o[:, W + 2:]
    res[:, W + N] = o[:, W]
    return res


def run(center_vec, windows_vecs, neg_vecs, buy_vec, trace: bool = False):
    """Run on 8 NeuronCores; returns (full_output, BassKernelResults)."""
    nc = _get_nc()
    in_maps = _shard_inputs(np.asarray(center_vec, dtype=np.float32),
                            np.asarray(windows_vecs, dtype=np.float32),
                            np.asarray(neg_vecs, dtype=np.float32),
                            np.asarray(buy_vec, dtype=np.float32))
    res = bass_utils.run_bass_kernel_spmd(
        nc, in_maps, list(range(NCORES)), trace=trace)
    full = np.concatenate(
        [_unpack_core(res.results[i]["out"]) for i in range(NCORES)], axis=0)
    return full.reshape(B, R, 1), res


def kernel(center_vec, windows_vecs, neg_vecs, buy_vec):
    out, _ = run(center_vec, windows_vecs, neg_vecs, buy_vec)
    return out
